# revision 27
# baseline (speedup 1.0000x reference)
"""Trainium2 Bass kernel for nn_BitfieldLinear (vq_codebook).

Reference computation:
    idx   = codes & 0xFF            (basis row, 256 entries)
    r_q   = (codes >> 8) & 0xFFF
    sign  = bit20 ? -1 : +1
    scale = sign * tanh(r_q / 4095)
    W     = scale[:, None] * basis[idx]        # [8192, 4096]
    y     = x @ W.T                            # [128, 8192]

Key factorization (never materialize the 128MB W):
    Z = x @ basis.T                            # [128, 256]  tiny matmul
    y[b, j] = scale[j] * Z[b, idx[j]]          # column gather + scale

Sorted-codes sharding: the host argsorts the 8192 codes by basis index
and hands each core a contiguous run of 1024 sorted codes.  Each run
spans only ~34 consecutive basis rows (KB=48 gives slack), so a core
needs a 48-row slice of basis instead of all 256 rows:

    ZT_c = basis[lo:lo+48] @ x.T               # [48, 128]  32 fp16 matmuls
    G_c[r, j] = scale[j] * (idx[j]-lo == r)    # [48, 1024] scaled one-hot
    y_c  = ZT_c.T @ G_c                        # [128, 1024] two 512-wide mms

The per-core DMA queues share one engine group (transfers serialize
with switch gaps and each transfer's data is only usable at its final
semaphore), so constants ride inside existing transfers: -lo and the
iota row travel as f32 bit patterns in the int32 codes tensor, and the
transpose identity leads the first basis chunk.  The tiny codes block
lands first so decode (DVE-only; tanh via odd minimax polynomial, rel
err 2e-4) overlaps the stream.  G is built in two wide stride-0
broadcast passes at a 64-column tile stride so pair transposes land
tile 2p+1 at partition 64 (legal quadrant), then cast to g16 [48,1024].
x^T is K-tiled across both HWDGE rings; Z accumulates in PSUM over 32
basis-stationary matmuls (Z lands pre-transposed).  The two output
halves cast and store on different engines/rings.  Host reassembles y
through the sort permutation (pure layout).
"""

import sys

for _p in ("/opt/trn_rl_repo", "/opt/pypackages"):
    if _p not in sys.path:
        sys.path.insert(0, _p)

import numpy as np

import concourse.bacc as bacc
import concourse.mybir as mybir
import concourse.tile as tile
from concourse.alu_op_type import AluOpType
from concourse.bass_utils import run_bass_kernel_spmd

N_CORES = 8
BATCH = 128
IN_F = 4096
OUT_F = 8192
BASIS = 256
OPC = OUT_F // N_CORES      # 1024 output columns per core
NK = IN_F // 128            # 32 K-tiles
NT = OPC // 128             # 8 code-tiles per core
KB = 48                     # basis rows per core (sorted span is ~34)
R_LEVELS = 4095.0
CW = NT + 1 + KB            # c128x columns: codes | -lo | iota

# tanh(r) ~ r*(C0 + C1 t + C2 t^2 + C3 t^3), t = r^2: minimax on [0,1],
# max rel err 2.0e-4
C0, C1, C2, C3 = 0.9999357544872516, -0.3310488478400793, \
    0.12016162322709638, -0.027606003207870822

F32 = mybir.dt.float32
FP16 = mybir.dt.float16
I32 = mybir.dt.int32

X_SCALAR_CHUNKS = [(0, 7), (7, 14), (14, 20)]
X_SYNC_CHUNKS = [(20, 26), (26, 32)]
B_CHUNKS = [(0, 8), (8, 20), (20, 32)]
NB0 = B_CHUNKS[0][1]        # basis tiles in the first (ident-led) chunk
B0COLS = 128 + NB0 * KB     # identity | basis tiles [0, NB0)


def build_nc():
    nc = bacc.Bacc(
        "TRN2",
        target_bir_lowering=False,
        debug=False,
        num_devices=N_CORES,
    )

    c128_d = nc.dram_tensor("c128", [128, CW], I32, kind="ExternalInput")
    b16_ds = [
        nc.dram_tensor(
            f"b16c{i}",
            [128, B0COLS if i == 0 else (be - bs) * KB],
            FP16, kind="ExternalInput")
        for i, (bs, be) in enumerate(B_CHUNKS)
    ]
    xs_ds = [
        nc.dram_tensor(f"x16s{i}", [128, (xe - xs) * 128], FP16,
                       kind="ExternalInput")
        for i, (xs, xe) in enumerate(X_SCALAR_CHUNKS)
    ]
    xy_ds = [
        nc.dram_tensor(f"x16y{i}", [128, (xe - xs) * 128], FP16,
                       kind="ExternalInput")
        for i, (xs, xe) in enumerate(X_SYNC_CHUNKS)
    ]
    out_d = nc.dram_tensor("out", [128, OPC], FP16, kind="ExternalOutput")

    with tile.TileContext(nc) as tc:
        with (
            tc.tile_pool(name="pool", bufs=1) as pool,
            tc.tile_pool(name="zps", bufs=1, space="PSUM") as zps,
            tc.tile_pool(name="tps", bufs=2, space="PSUM") as tps,
            tc.tile_pool(name="yps", bufs=1, space="PSUM") as yps,
        ):
            # ---- sync ring: codes+consts first (tiny), then the basis
            # slice (identity leads chunk 0), then the tail of x^T.
            # tile_wait_until feeds the compile-time scheduler realistic
            # completion times for each transfer (the shared DMA engine
            # group serializes transfers), so it assigns minimal sem waits
            # instead of assuming all data lands instantly.
            c128 = pool.tile([128, CW], I32)
            with tc.tile_wait_until(0.0019):
                nc.sync.dma_start(out=c128[:], in_=c128_d[:])
            b0_sb = pool.tile([128, B0COLS], FP16)
            with tc.tile_wait_until(0.0031):
                nc.sync.dma_start(out=b0_sb[:], in_=b16_ds[0][:])
            b1_sb = pool.tile([128, (B_CHUNKS[1][1] - B_CHUNKS[1][0]) * KB],
                              FP16)
            with tc.tile_wait_until(0.0040):
                nc.sync.dma_start(out=b1_sb[:], in_=b16_ds[1][:])
            b2_sb = pool.tile([128, (B_CHUNKS[2][1] - B_CHUNKS[2][0]) * KB],
                              FP16)
            with tc.tile_wait_until(0.0050):
                nc.sync.dma_start(out=b2_sb[:], in_=b16_ds[2][:])
            x16_sb = pool.tile([128, IN_F], FP16)
            for i, (xs, xe) in enumerate(X_SYNC_CHUNKS):
                with tc.tile_wait_until(0.0058 + 0.0007 * i):
                    nc.sync.dma_start(
                        out=x16_sb[:, xs * 128:xe * 128], in_=xy_ds[i][:]
                    )

            # ---- scalar ring: the head of x^T
            for i, (xs, xe) in enumerate(X_SCALAR_CHUNKS):
                with tc.tile_wait_until(0.0027 + 0.0013 * i):
                    nc.scalar.dma_start(
                        out=x16_sb[:, xs * 128:xe * 128], in_=xs_ds[i][:]
                    )
            # absorb any ACT table load while the engine is idle
            dummy = pool.tile([128, 1], F32, name="dummy")
            nc.vector.memset(dummy[:], 0.0)
            dummy2 = pool.tile([128, 1], F32, name="dummy2")
            nc.scalar.copy(out=dummy2[:], in_=dummy[:])

            ident = b0_sb[:, 0:128]
            iota_v = c128[:, NT + 1:CW].bitcast(F32)

            def b_tile(n):
                if n < NB0:
                    return b0_sb[:, 128 + n * KB:128 + (n + 1) * KB]
                if n < B_CHUNKS[1][1]:
                    m = n - B_CHUNKS[1][0]
                    return b1_sb[:, m * KB:(m + 1) * KB]
                m = n - B_CHUNKS[2][0]
                return b2_sb[:, m * KB:(m + 1) * KB]

            # ---- decode on DVE: idx_local (f32), scale (f32), [128, NT]
            idx_i = pool.tile([128, NT], I32, name="idx_i")
            nc.vector.tensor_scalar(
                out=idx_i[:], in0=c128[:, 0:NT],
                scalar1=255, scalar2=None, op0=AluOpType.bitwise_and,
            )
            idx_f = pool.tile([128, NT], F32, name="idx_f")
            nc.vector.tensor_scalar(
                out=idx_f[:], in0=idx_i[:],
                scalar1=1.0, scalar2=c128[:, NT:NT + 1].bitcast(F32),
                op0=AluOpType.mult, op1=AluOpType.add,
            )
            rq_i = pool.tile([128, NT], I32, name="rq_i")
            nc.vector.tensor_scalar(
                out=rq_i[:], in0=c128[:, 0:NT],
                scalar1=8, scalar2=4095,
                op0=AluOpType.logical_shift_right,
                op1=AluOpType.bitwise_and,
            )
            r = pool.tile([128, NT], F32, name="r")
            nc.vector.tensor_scalar_mul(
                out=r[:], in0=rq_i[:], scalar1=1.0 / R_LEVELS
            )
            sg_i = pool.tile([128, NT], I32, name="sg_i")
            nc.vector.tensor_scalar(
                out=sg_i[:], in0=c128[:, 0:NT],
                scalar1=20, scalar2=1,
                op0=AluOpType.logical_shift_right,
                op1=AluOpType.bitwise_and,
            )
            rs = pool.tile([128, NT], F32, name="rs")
            nc.vector.tensor_scalar(
                out=rs[:], in0=sg_i[:],
                scalar1=-2.0, scalar2=1.0,
                op0=AluOpType.mult, op1=AluOpType.add,
            )
            nc.vector.tensor_tensor(
                out=rs[:], in0=rs[:], in1=r[:], op=AluOpType.mult
            )
            t2 = pool.tile([128, NT], F32, name="t2")
            nc.vector.tensor_tensor(
                out=t2[:], in0=r[:], in1=r[:], op=AluOpType.mult
            )
            h = pool.tile([128, NT], F32, name="h")
            nc.vector.tensor_scalar(
                out=h[:], in0=t2[:], scalar1=C3, scalar2=C2,
                op0=AluOpType.mult, op1=AluOpType.add,
            )
            nc.vector.tensor_tensor(
                out=h[:], in0=h[:], in1=t2[:], op=AluOpType.mult
            )
            nc.vector.tensor_scalar(
                out=h[:], in0=h[:], scalar1=C1, scalar2=None,
                op0=AluOpType.add,
            )
            nc.vector.tensor_tensor(
                out=h[:], in0=h[:], in1=t2[:], op=AluOpType.mult
            )
            nc.vector.tensor_scalar(
                out=h[:], in0=h[:], scalar1=C0, scalar2=None,
                op0=AluOpType.add,
            )
            scl = pool.tile([128, NT], F32, name="scl")
            nc.vector.tensor_tensor(
                out=scl[:], in0=h[:], in1=rs[:], op=AluOpType.mult
            )

            # ---- G^T in two wide broadcast passes per half:
            # gt[j, (t, k)] = (iota[k] == idx_l[t*128+j]) * scl[t*128+j]
            # tiles sit at 64-column stride (48 data + 16 zero pad) so the
            # pair transposes land tile 2p+1 at partition 64 (legal read)
            half = NT // 2
            gt_all = pool.tile([128, NT * 64], FP16, name="gt_all")
            gt3 = gt_all[:].rearrange("p (t k) -> p t k", k=64)
            nc.vector.memset(gt3[:, :, KB:64], 0.0)
            eq = pool.tile([128, half * KB], FP16, name="eq")
            for hh in range(2):
                ts = hh * half
                io_v = iota_v.unsqueeze(1).broadcast_to([128, half, KB])
                idx_v = idx_f[:, ts:ts + half].unsqueeze(2).broadcast_to(
                    [128, half, KB]
                )
                scl_v = scl[:, ts:ts + half].unsqueeze(2).broadcast_to(
                    [128, half, KB]
                )
                eq_v = eq[:].rearrange("p (t k) -> p t k", k=KB)
                nc.vector.tensor_tensor(
                    out=eq_v, in0=io_v, in1=idx_v, op=AluOpType.is_equal
                )
                gt_v = gt3[:, ts:ts + half, 0:KB]
                nc.vector.tensor_tensor(
                    out=gt_v, in0=eq_v, in1=scl_v, op=AluOpType.mult
                )

            # ---- ZT [48, 128] += basis_tile^T @ x_tile over 32 K-tiles
            zt_ps = zps.tile([KB, 128], F32, tag="z")
            for n in range(NK):
                nc.tensor.matmul(
                    zt_ps[:],
                    lhsT=b_tile(n),
                    rhs=x16_sb[:, n * 128:(n + 1) * 128],
                    start=(n == 0), stop=(n == NK - 1),
                )
            zt16 = pool.tile([KB, 128], FP16, name="zt16")
            nc.vector.tensor_copy(out=zt16[:], in_=zt_ps[:])

            # ---- transpose G in pairs ([128, 128] -> [128, 128], two tiles
            # per pass at partitions 0 and 64) and cast into g16 [48, 1024]
            g16 = pool.tile([KB, OPC], FP16, name="g16")
            for p in range(NT // 2):
                tp = tps.tile([128, 128], FP16, tag="tp", name=f"tp{p}")
                nc.tensor.transpose(
                    out=tp[:], in_=gt_all[:, p * 128:(p + 1) * 128],
                    identity=ident,
                )
                for s in range(2):
                    t = 2 * p + s
                    nc.vector.tensor_copy(
                        out=g16[:, t * 128:(t + 1) * 128],
                        in_=tp[s * 64:s * 64 + KB, :],
                    )

            # ---- y = ZT.T @ G, two 512-wide fp16 matmuls; the two halves
            # cast and store on different engines/rings
            y_sbs = []
            for nch in range(2):
                y_ps = yps.tile([128, 512], F32, tag=f"y{nch}", name=f"y_ps{nch}")
                nc.tensor.matmul(
                    y_ps[:],
                    lhsT=zt16[:],
                    rhs=g16[:, nch * 512:(nch + 1) * 512],
                    start=True, stop=True,
                )
                y_sb = pool.tile([128, 512], FP16, tag=f"ysb{nch}", name=f"y_sb{nch}")
                if nch == 0:
                    nc.scalar.copy(out=y_sb[:], in_=y_ps[:])
                else:
                    nc.vector.tensor_copy(out=y_sb[:], in_=y_ps[:])
                y_sbs.append(y_sb)
            nc.scalar.dma_start(out=out_d[:, 0:512], in_=y_sbs[0][:])
            nc.sync.dma_start(out=out_d[:, 512:1024], in_=y_sbs[1][:])

    nc.compile()
    return nc


_NC = None


def _get_nc():
    global _NC
    if _NC is None:
        _NC = build_nc()
    return _NC


def make_in_maps(x, codes, basis):
    x = np.ascontiguousarray(x, dtype=np.float32)
    basis = np.ascontiguousarray(basis, dtype=np.float32)
    codes = np.ascontiguousarray(codes, dtype=np.int32)

    # xt[p, n*128 + m] = x[m, n*128 + p]
    xt = (
        x.reshape(BATCH, NK, 128).transpose(2, 1, 0).reshape(128, IN_F)
    ).astype(np.float16)
    shared = {}
    for pre, chunks in (("x16s", X_SCALAR_CHUNKS), ("x16y", X_SYNC_CHUNKS)):
        for i, (xs, xe) in enumerate(chunks):
            shared[f"{pre}{i}"] = np.ascontiguousarray(
                xt[:, xs * 128:xe * 128]
            )
    iota_bits = np.arange(KB, dtype=np.float32).view(np.int32)

    # sort codes by basis index; each core gets 1024 consecutive sorted
    # codes whose indices span < KB consecutive basis rows
    idx = codes & 255
    order = np.argsort(idx, kind="stable")
    in_maps = []
    sels = []
    for c in range(N_CORES):
        sel = order[c * OPC:(c + 1) * OPC]
        sels.append(sel)
        csort = codes[sel]
        lo = int(idx[sel].min())
        span = int(idx[sel].max()) - lo + 1
        if span > KB:
            raise ValueError(f"core {c}: sorted idx span {span} > KB={KB}")
        # c128x: wrap-128 codes | -lo (f32 bits) | iota row (f32 bits)
        c128 = np.empty((128, CW), dtype=np.int32)
        c128[:, :NT] = csort.reshape(NT, 128).T
        c128[:, NT] = np.float32(-lo).view(np.int32)
        c128[:, NT + 1:] = iota_bits[None, :]
        # basis slice rows [lo, lo+KB) zero-padded past row 255;
        # bt[p, n*KB + r] = basis[lo + r, n*128 + p]; chunk 0 is led by a
        # 128-col fp16 identity (transpose operand)
        sl = np.zeros((KB, IN_F), dtype=np.float32)
        avail = min(KB, BASIS - lo)
        sl[:avail] = basis[lo:lo + avail]
        bt = (
            sl.reshape(KB, NK, 128).transpose(2, 1, 0).reshape(128, NK * KB)
        ).astype(np.float16)
        b0 = np.empty((128, B0COLS), dtype=np.float16)
        b0[:, 0:128] = np.eye(128, dtype=np.float16)
        b0[:, 128:] = bt[:, :NB0 * KB]
        m = {"c128": np.ascontiguousarray(c128),
             "b16c0": np.ascontiguousarray(b0), **shared}
        for i, (bs, be) in enumerate(B_CHUNKS[1:], start=1):
            m[f"b16c{i}"] = np.ascontiguousarray(bt[:, bs * KB:be * KB])
        in_maps.append(m)
    return in_maps, sels


def assemble_output(results, sels):
    y = np.empty((BATCH, OUT_F), dtype=np.float32)
    for c in range(N_CORES):
        y[:, sels[c]] = results[c]["out"].astype(np.float32)
    return y


def kernel(x, codes, basis):
    nc = _get_nc()
    in_maps, sels = make_in_maps(x, codes, basis)
    res = run_bass_kernel_spmd(nc, in_maps, list(range(N_CORES)))
    return assemble_output(res.results, sels)


if __name__ == "__main__":
    rng = np.random.default_rng(0)
    x = rng.standard_normal((BATCH, IN_F), dtype=np.float32)
    basis = (rng.standard_normal((BASIS, IN_F)) * 0.02).astype(np.float32)
    codes = rng.integers(0, 1 << 22, size=(OUT_F,), dtype=np.int32)
    y = kernel(x, codes, basis)

    idx = codes & 255
    r = ((codes >> 8) & 4095).astype(np.float32) / R_LEVELS
    sign = np.where(((codes >> 20) & 1) == 1, -1.0, 1.0).astype(np.float32)
    scale = sign * np.tanh(r)
    W = scale[:, None] * basis[idx]
    y_ref = x @ W.T
    err = np.linalg.norm(y - y_ref) / np.linalg.norm(y_ref)
    print("rel err:", err)
